# revision 57
# baseline (speedup 1.0000x reference)
"""BotRGCN (2-layer relational GCN) Trainium2 kernel, 8-way SPMD.

Strategy (per sharding hint): nodes sharded contiguously across 8 cores;
edges partitioned by destination core; relation weights replicated.

Per core, per RGCN layer, for each 128-destination-node tile we accumulate
S^T[fi, c] (c = rel*128 + local_dst for rel 0..2; root self-loop occupies
columns [384, 512) via an identity-matmul transpose of the tile's own h
rows) in PSUM via one-hot matmuls: gather source rows h[src] with SWDGE
dma_gather (bf16, 256B rows), build narrow one-hot A matrices on DVE
(iota==key)*norm, and let the tensor engine do the scatter-add:
S^T += E^T @ A.  Then 4 weight matmuls (3 relations + root) + bias produce
the tile's output.  Feature MLP (fc1/fc2/concat/leaky-relu) and the final
fc3 also run on device; h and h1 are AllGathered piece-wise (pipelined
behind compute) so every core can gather any source row.  The host only
shards/packs inputs, plans edge slots, and indexes the final logits.

Self-contained: only imports the system concourse toolchain.
"""
import os
import sys

for _p in ("/opt/trn_rl_repo", "/root/.axon_site/_ro/trn_rl_repo"):
    if os.path.isdir(_p) and _p not in sys.path:
        sys.path.insert(0, _p)

import numpy as np
import ml_dtypes

from concourse import bass, bacc, tile, mybir
from concourse.bass_utils import run_bass_kernel_spmd

BF16 = ml_dtypes.bfloat16

# ---------------- problem constants (hardcoded per spec) ----------------
N_NODES = 50000
N_REL = 3
FEAT = 128
VAL = 16
TEXT = 768
CLASSES = 2
CORES = 8
P = 128           # partition / tile size
W = 96            # one-hot window width
CTMAX = N_REL * P  # 384: relation-major ct space for real edges
CHMAX = 8         # slots per gather chunk (1024 idxs = SWDGE ring cap)
ABATCH = 32       # slots per A-matrix build batch
RSLOT = 4         # 3 relations + root self-loop
NPIECE = 2        # pipelined AllGather pieces (= gather streams)


# ============================ host planner =============================

def _build_schedule(cts, cmax):
    """Joint (cross-core) slot schedule for one (tile, section).

    cts: list of 8 sorted int arrays (edge keys in [0, cmax)).
    Returns (bases, ranges) where bases[j] is the shared window base of
    slot j and ranges[c][j] = (start, end) into core c's sorted arrays.
    """
    n = len(cts)
    ptrs = [0] * n
    lens = [len(a) for a in cts]
    bases = []
    ranges = [[] for _ in range(n)]
    while any(ptrs[c] < lens[c] for c in range(n)):
        b = min(cts[c][ptrs[c]] for c in range(n) if ptrs[c] < lens[c])
        b = min(int(b), cmax - W)
        bases.append(b)
        for c in range(n):
            s = ptrs[c]
            hi = int(np.searchsorted(cts[c], b + W, side="left"))
            e = min(s + P, hi)
            e = max(e, s)
            ranges[c].append((s, e))
            ptrs[c] = e
    return bases, ranges


class Plan:
    pass


def _assign_buckets(nodes, din, nbuckets, cap):
    """Greedy 2-D balanced assignment of `nodes` (sorted by load desc)
    into `nbuckets` buckets of `cap` nodes each, balancing both columns
    of din[nodes].  Returns [nbuckets, cap] node ids (-1 pad)."""
    tot = din[nodes].sum(0).astype(np.float64)
    order = np.argsort(-(din[nodes].sum(1)))
    b_load = np.zeros((nbuckets, 2))
    b_cnt = np.zeros(nbuckets, np.int64)
    out = np.full((nbuckets, cap), -1, np.int64)
    t0, t1 = tot / nbuckets
    for n in nodes[order]:
        d0, d1 = din[n]
        score = np.maximum((b_load[:, 0] + d0) / max(t0, 1),
                           (b_load[:, 1] + d1) / max(t1, 1))
        score[b_cnt >= cap] = np.inf
        b = int(np.argmin(score))
        out[b, b_cnt[b]] = n
        b_load[b, 0] += d0
        b_load[b, 1] += d1
        b_cnt[b] += 1
    return out


def make_plan(edge_index, edge_type, n_nodes=N_NODES, cores=CORES, lolim=None):
    """Edge partition + joint slot schedule shared by both RGCN layers."""
    pl = Plan()
    pl.cores = cores
    NS = n_nodes // cores
    assert NS * cores == n_nodes
    # 50 tiles: two equal 25-tile pieces so both gather streams carry
    # ~the same edge load (crucial for the 6+6 slots/tile packing floor).
    NT = 2 * ((((NS + P - 1) // P) + 1) // 2)
    NSP = NT * P
    NROWS = cores * NSP
    pl.NS, pl.NSP, pl.NT, pl.NROWS = NS, NSP, NT, NROWS

    # h rows live in NPIECE=2 half-tensors (piece p covers local tiles
    # [tb[p], tb[p+1])), each AllGathered by a single collective and
    # small enough (<32768 rows) for int16 gather indexing.  The gather
    # stream of an edge is the piece its SOURCE row lives in.
    tb = np.linspace(0, NT, NPIECE + 1).round().astype(np.int64)
    prow = (tb[1:] - tb[:-1]) * P          # local rows per piece
    assert (prow * cores < 32768).all()
    pl.piece_tb, pl.piece_rows = tb, prow

    src = np.asarray(edge_index[0], np.int64)
    dst = np.asarray(edge_index[1], np.int64)
    arel = np.asarray(edge_type, np.int64)

    deg = np.zeros((N_REL, n_nodes), np.int64)
    np.add.at(deg, (arel, dst), 1)
    anorm = 1.0 / np.maximum(deg[arel, dst], 1).astype(np.float32)

    # ---- balanced node->(core,tile) permutation ----
    # Split nodes into two equal pools (pool = which h half their row
    # lands in); an edge's gather stream is then pool(src), fixed before
    # bucket placement.  Buckets (core,tile) of 128 rows get cap real
    # nodes each, chosen to balance (indeg-from-pool0, indeg-from-pool1).
    half = n_nodes // 2
    indeg = np.zeros(n_nodes, np.int64)
    np.add.at(indeg, dst, 1)
    bydeg = np.argsort(-indeg, kind="stable")
    pool_nodes = [np.sort(bydeg[0::2]), np.sort(bydeg[1::2])]
    in_pool1 = np.zeros(n_nodes, bool)
    in_pool1[pool_nodes[1]] = True
    din = np.zeros((n_nodes, 2), np.int64)
    np.add.at(din, (dst, in_pool1[src].astype(np.int64)), 1)

    htiles = NT // 2
    cap = -(-NS // NT)          # real nodes per bucket (e.g. 125)
    assert cap * htiles >= len(pool_nodes[0]) // cores
    node_of_row = np.full((cores, NSP), -1, np.int64)
    for pI in (0, 1):
        nb = cores * htiles
        bk = _assign_buckets(pool_nodes[pI], din, nb, cap)
        # bucket b -> core b % cores, tile tb[pI] + b // cores
        for b in range(nb):
            c = b % cores
            t = int(tb[pI]) + b // cores
            ids = bk[b][bk[b] >= 0]
            node_of_row[c, t * P:t * P + len(ids)] = ids
    pl.node_of_row = node_of_row

    # node -> (core, local row)
    core_of = np.zeros(n_nodes, np.int64)
    row_of = np.zeros(n_nodes, np.int64)
    for c in range(cores):
        m = node_of_row[c] >= 0
        core_of[node_of_row[c][m]] = c
        row_of[node_of_row[c][m]] = np.nonzero(m)[0]

    t_src = row_of[src] // P
    pc = np.searchsorted(tb, t_src, side="right") - 1
    row = core_of[src] * prow[pc] + (row_of[src] - tb[pc] * P)
    sec = pc                                         # gather stream

    owner = core_of[dst]
    loc = row_of[dst]
    tile_id = loc // P
    ct = arel * P + (loc % P)

    order = np.lexsort((ct, sec, tile_id, owner))
    row, ct, sec, anorm = row[order], ct[order], sec[order], anorm[order]
    owner, tile_id = owner[order], tile_id[order]

    # index boundaries for (core, tile, sec) groups
    key = (owner * NT + tile_id) * 2 + sec
    bounds = np.searchsorted(key, np.arange(cores * NT * 2 + 1))

    def group(c, t, s):
        k = (c * NT + t) * 2 + s
        return bounds[k], bounds[k + 1]

    # per (tile, sec): joint schedule; accumulate per-core slot data
    slot_tile = {0: [], 1: []}      # per section stream: tile of each slot
    slot_base = {0: [], 1: []}
    idx16 = {0: [[] for _ in range(cores)], 1: [[] for _ in range(cores)]}
    keyd = {0: [[] for _ in range(cores)], 1: [[] for _ in range(cores)]}
    nrmd = {0: [[] for _ in range(cores)], 1: [[] for _ in range(cores)]}
    tile_slot_range = {0: np.zeros((NT, 2), np.int64), 1: np.zeros((NT, 2), np.int64)}

    for t in range(NT):
        for s in (0, 1):
            cts, rows_, nrms_ = [], [], []
            for c in range(cores):
                a, b = group(c, t, s)
                cts.append(ct[a:b])
                rows_.append(row[a:b])
                nrms_.append(anorm[a:b])
            start = len(slot_base[s])
            bases, ranges = _build_schedule(cts, CTMAX)
            for j, bj in enumerate(bases):
                slot_tile[s].append(t)
                slot_base[s].append(bj)
            for c in range(cores):
                for j, (a, b) in enumerate(ranges[c]):
                    n = b - a
                    ii = np.zeros(P, np.int16)
                    kk = np.full(P, -1.0, np.float32)
                    nn = np.zeros(P, np.float32)
                    if n > 0:
                        ii[:n] = rows_[c][a:b].astype(np.int16)
                        kk[:n] = (cts[c][a:b] - bases[j]).astype(np.float32)
                        nn[:n] = nrms_[c][a:b]
                        # pad with copies of real rows (key=-1 keeps A=0):
                        # all-zero padding would hammer one DRAM row and
                        # serialize the gather DMA on a single bank.
                        reps = rows_[c][a:b][np.arange(P - n) % n]
                        ii[n:] = reps.astype(np.int16)
                    else:
                        lim = cores * int(prow[s])
                        ii[:] = ((np.arange(P) * 977 + j * 131) % lim).astype(
                            np.int16)
                    idx16[s][c].append(ii)
                    keyd[s][c].append(kk)
                    nrmd[s][c].append(nn)
            tile_slot_range[s][t] = (start, len(slot_base[s]))

    pl.NLO = len(slot_base[0])
    pl.NHI = len(slot_base[1])
    pl.NSLOT = pl.NLO + pl.NHI
    pl.slot_base = {s: np.array(slot_base[s], np.int64) for s in (0, 1)}
    pl.slot_tile = {s: np.array(slot_tile[s], np.int64) for s in (0, 1)}
    pl.tile_slot_range = tile_slot_range

    # per-core packed arrays
    pl.idx_wrapped = {}
    pl.keys = {}
    pl.norms = {}
    for c in range(cores):
        parts = []
        for s in (0, 1):
            arr = (np.stack(idx16[s][c]) if idx16[s][c]
                   else np.zeros((0, P), np.int16))
            parts.append(arr)
        pl.idx_wrapped[c] = parts  # list of [nslot, 128] int16 per section
        kk = np.concatenate(
            [np.stack(keyd[s][c]) if keyd[s][c] else np.zeros((0, P), np.float32)
             for s in (0, 1)])
        nn = np.concatenate(
            [np.stack(nrmd[s][c]) if nrmd[s][c] else np.zeros((0, P), np.float32)
             for s in (0, 1)])
        pl.keys[c] = np.ascontiguousarray(kk.T.astype(BF16))    # [128, NSLOT]
        pl.norms[c] = np.ascontiguousarray(nn.T.astype(BF16))   # [128, NSLOT]

    # gather chunks per section stream: list of (s0, ns)
    pl.chunks = {}
    for s in (0, 1):
        n = [pl.NLO, pl.NHI][s]
        ch = []
        i = 0
        while i < n:
            ns = min(CHMAX, n - i)
            ch.append((i, ns))
            i += ns
        pl.chunks[s] = ch
    return pl


def wrap16(flat):
    """[L] int16 -> [128, L//16] wrapped layout for dma_gather idxs."""
    L = len(flat)
    assert L % 16 == 0
    a = np.asarray(flat, np.int16).reshape(-1, 16).T  # [16, L//16]
    return np.ascontiguousarray(np.tile(a, (8, 1)))



def blob_layout(pl):
    """Ordered (name, nelem, shape) segments of the single bf16 input blob.
    int16 segments are stored bit-cast as bf16. Offsets 128-elem aligned."""
    NSP, NT, NSLOT = pl.NSP, pl.NT, pl.NSLOT
    NLO, NHI = pl.NLO, pl.NHI
    TC = TEXT // P
    segs = [
        ("textT", [NT, P, TC * P]),
        ("valT", [VAL, NSP]),
        ("fwv", [VAL, FEAT]),
        ("fwt", [P, TC * P]),
        ("beff", [1, FEAT]),
        ("ww1", [P, RSLOT * FEAT]),
        ("b1", [1, FEAT]),
        ("ww2", [P, RSLOT * FEAT]),
        ("b2", [1, FEAT]),
        ("fc3w", [FEAT, CLASSES]),
        ("fc3b", [1, CLASSES]),
        ("iota16", [P, ABATCH * W]),
        ("ones1", [1, P]),
        ("ident", [P, P]),
        ("keys", [P, max(NSLOT, 1)]),
        ("norms", [P, max(NSLOT, 1)]),
        ("idxlo", [P, max(NLO, 1) * 8]),
        ("idxhi", [P, max(NHI, 1) * 8]),
    ]
    out = {}
    off = 0
    for name, shape in segs:
        n = int(np.prod(shape))
        out[name] = (off, n, shape)
        off += ((n + 127) // 128) * 128
    return out, off

# ============================ bass builder =============================

def build_bass(pl, ablate=()):
    ab = set(ablate)
    NSP, NT = pl.NSP, pl.NT
    NROWS = pl.NROWS
    NLO, NHI, NSLOT = pl.NLO, pl.NHI, pl.NSLOT
    TC = TEXT // P  # text chunks

    cores = getattr(pl, "cores", CORES)
    nc = bacc.Bacc("TRN2", target_bir_lowering=False, debug=False,
                   num_devices=cores, num_swdge_queues=4,
                   dynamic_dma_scratch_size=32768)
    qrr = {"n": 0}  # round-robin SWDGE queue picker
    dt = mybir.dt
    f32, bf, i16 = dt.float32, dt.bfloat16, dt.int16

    # ---- parameters: one packed bf16 blob + output
    layout, blob_n = blob_layout(pl)
    p_blob = nc.declare_dram_parameter("blob", [1, blob_n], bf, isOutput=False)
    p_logT = nc.declare_dram_parameter("logitsT", [CLASSES, NSP], f32, isOutput=True)

    def seg(name, dtype=bf):
        off, n, shape = layout[name]
        ap = p_blob[0:1, off:off + n]
        if dtype != bf:
            ap = ap.bitcast(dtype)
        r = int(np.prod(shape[:-1]))
        return ap.rearrange("o (r c) -> (o r) c", r=r)

    use_coll = cores > 1 and "coll" not in ab

    with tile.TileContext(nc) as tc:
        with tc.tile_pool(name="wt", bufs=1) as wt, \
             tc.tile_pool(name="sb", bufs=2) as sb, \
             tc.tile_pool(name="elo", bufs=20) as elo, \
             tc.tile_pool(name="ehi", bufs=10) as ehi, \
             tc.tile_pool(name="alo", bufs=6) as alo, \
             tc.tile_pool(name="ahi", bufs=6) as ahi, \
             tc.tile_pool(name="tts", bufs=3) as tts, \
             tc.tile_pool(name="dram", bufs=1, space="DRAM") as dram:

            # ---- resident weights / tables
            def resident(name, dtype=bf):
                off, n, shape = layout[name]
                t = wt.tile(list(shape[-2:] if len(shape) == 2 else shape), dtype,
                            tag=name)
                nc.sync.dma_start(t[:], seg(name, dtype))
                return t

            fwv = resident("fwv")
            fwt = resident("fwt")
            beff = resident("beff")
            ww1 = resident("ww1")
            b1 = resident("b1")
            ww2 = resident("ww2")
            b2 = resident("b2")
            fc3w = resident("fc3w")
            fc3b = resident("fc3b")
            iota16 = resident("iota16")
            ones1 = resident("ones1")
            ident = resident("ident")
            valT = resident("valT")
            keys = resident("keys")
            norms = resident("norms")
            idxsb = [resident("idxlo", i16), resident("idxhi", i16)]

            # ---- DRAM intermediates
            h_shard = dram.tile([NSP, FEAT], bf)
            _as = "Shared" if use_coll else "Local"
            h_half = [dram.tile([cores * int(pl.piece_rows[p]), FEAT], bf,
                                addr_space=_as, name=f"hh{p}")
                      for p in range(NPIECE)]
            h1_shard = dram.tile([NSP, FEAT], bf)
            h1_half = [dram.tile([cores * int(pl.piece_rows[p]), FEAT], bf,
                                 addr_space=_as, name=f"h1h{p}")
                       for p in range(NPIECE)]
            warm_in = dram.tile([1, P], bf)
            warm_out = dram.tile([cores, P], bf, addr_space=_as)

            def ag_piece(src_shard, dst_halves, p):
                t0, t1 = int(pl.piece_tb[p]), int(pl.piece_tb[p + 1])
                if use_coll:
                    # high_priority: the trigger must not queue behind
                    # prefetched gathers in the gpsimd engine stream.
                    with tc.high_priority():
                        nc.gpsimd.collective_compute(
                            "AllGather", mybir.AluOpType.bypass,
                            replica_groups=[list(range(cores))],
                            ins=[src_shard[t0 * P:t1 * P, :].opt()],
                            outs=[dst_halves[p][:].opt()])
                else:
                    nc.sync.dma_start(dst_halves[p][:],
                                      src_shard[t0 * P:t1 * P, :])

            # warmup collective: absorbs cross-core launch skew while the
            # MLP computes (no data deps, so it schedules immediately).
            if use_coll:
                nc.sync.dma_start(warm_in[0:1, :], ones1[:])
                nc.gpsimd.collective_compute(
                    "AllGather", mybir.AluOpType.bypass,
                    replica_groups=[list(range(cores))],
                    ins=[warm_in[:].opt()], outs=[warm_out[:].opt()])

            # ================= phase 1: feature MLP =================
            piece_of_tile = np.searchsorted(pl.piece_tb, np.arange(NT),
                                            side="right") - 1
            with tc.tile_pool(name="ps1", bufs=2, space="PSUM") as ps1:
                for t in range(NT):
                    tt = tts.tile([P, TC, P], bf, tag="tt")
                    toff = layout["textT"][0] + t * P * TC * P
                    nc.sync.dma_start(
                        tt[:], p_blob[0:1, toff:toff + P * TC * P]
                        .rearrange("o (p c n) -> (o p) c n", p=P, c=TC))
                    ph = ps1.tile([P, P], f32, tag="ph", space="PSUM")
                    nc.tensor.matmul(out=ph[:],
                                     lhsT=valT[:, t * P:(t + 1) * P],
                                     rhs=fwv[:],
                                     start=True, stop=False)
                    for c in range(TC):
                        nc.tensor.matmul(out=ph[:],
                                         lhsT=tt[:, c, :],
                                         rhs=fwt[:, c * P:(c + 1) * P],
                                         start=False, stop=False)
                    nc.tensor.matmul(out=ph[:], lhsT=ones1[:], rhs=beff[:],
                                     start=False, stop=True)
                    hsb = sb.tile([P, P], bf, tag="hsb")
                    nc.scalar.activation(out=hsb[:], in_=ph[:],
                                         func=mybir.ActivationFunctionType.Lrelu,
                                         alpha=0.01)
                    nc.sync.dma_start(h_shard[t * P:(t + 1) * P, :], hsb[:])
                    if t + 1 == pl.piece_tb[piece_of_tile[t] + 1]:
                        ag_piece(h_shard, h_half, int(piece_of_tile[t]))

            # ================= RGCN layers =================
            def rgcn_layer(src_shard, src_halves, ww, bb, layer, out_shard,
                           out_halves):
                emitted = {0: -1, 1: -1}   # last emitted gather chunk per stream
                aemitted = {0: -1, 1: -1}  # last emitted A batch per stream
                ebufs = {0: {}, 1: {}}     # chunk id -> (E tile, s0, ns)
                abufs = {0: {}, 1: {}}     # batch id -> (A tile, s0, ns)
                epools = {0: elo, 1: ehi}
                apools = {0: alo, 1: ahi}
                nstream = {0: NLO, 1: NHI}

                def emit_chunk(s, ci):
                    s0 = ci * CHMAX
                    ns = min(CHMAX, nstream[s] - s0)
                    et = epools[s].tile([P, CHMAX, FEAT], bf, tag=f"e{s}")
                    src_ap = src_halves[s][:]
                    if "gather" in ab:
                        nc.vector.memset(et[:, 0:1, 0:2], 0.0)
                        ebufs[s][ci] = (et, s0, ns)
                        ebufs[s].pop(ci - 9, None)
                        return
                    qrr["n"] += 1
                    nq = 1 if "oneq" in ab else 4
                    nc.gpsimd.dma_gather(
                        out_ap=et[:, 0:ns, :],
                        in_ap=src_ap,
                        idxs_ap=idxsb[s][:, s0 * 8:(s0 + ns) * 8],
                        num_idxs=ns * P,
                        num_idxs_reg=ns * P,
                        elem_size=FEAT,
                        queue_num=qrr["n"] % nq)
                    ebufs[s][ci] = (et, s0, ns)
                    ebufs[s].pop(ci - 9, None)

                def emit_abatch(s, ai):
                    s0 = ai * ABATCH
                    ns = min(ABATCH, nstream[s] - s0)
                    at = apools[s].tile([P, ABATCH, W], bf, tag=f"a{s}")
                    if "abuild" in ab:
                        nc.vector.memset(at[:, 0:1, 0:2], 0.0)
                        abufs[s][ai] = (at, s0, ns)
                        abufs[s].pop(ai - 5, None)
                        return
                    g0 = s0 + (0 if s == 0 else NLO)
                    kb = keys[:, g0:g0 + ns].unsqueeze(2).to_broadcast([P, ns, W])
                    nb = norms[:, g0:g0 + ns].unsqueeze(2).to_broadcast([P, ns, W])
                    ib = iota16[:, 0:ns * W].rearrange("p (n w) -> p n w", n=ns)
                    nc.vector.tensor_tensor(out=at[:, 0:ns, :], in0=ib, in1=kb,
                                            op=mybir.AluOpType.is_equal)
                    nc.vector.tensor_tensor(out=at[:, 0:ns, :],
                                            in0=at[:, 0:ns, :],
                                            in1=nb, op=mybir.AluOpType.mult)
                    abufs[s][ai] = (at, s0, ns)
                    abufs[s].pop(ai - 5, None)

                with tc.tile_pool(name=f"psl{layer}", bufs=2, space="PSUM") as psl:
                    for t in range(NT):
                        pS = psl.tile([P, RSLOT * P], f32, tag="pS", space="PSUM",
                                      bufs=3)
                        if "memset" not in ab:
                            nc.vector.memset(pS[:, 0:CTMAX], 0.0)
                        # root self-loop: S^T[:, 384:512] = h_tile^T via
                        # identity matmul (start=True zeroes that region).
                        hself = sb.tile([P, P], bf, tag="hself")
                        nc.sync.dma_start(hself[:],
                                          src_shard[t * P:(t + 1) * P, :])
                        nc.tensor.matmul(out=pS[:, CTMAX:CTMAX + P],
                                         lhsT=hself[:], rhs=ident[:],
                                         start=True, stop=False,
                                         skip_group_check=True)
                        for s in (0, 1):
                            a, b = pl.tile_slot_range[s][t]
                            for j in range(a, b):
                                ci = j // CHMAX
                                ai = j // ABATCH
                                if ci > emitted[s]:
                                    emit_chunk(s, ci)
                                    emitted[s] = ci
                                if ai > aemitted[s]:
                                    emit_abatch(s, ai)
                                    aemitted[s] = ai
                                et, es0, _ = ebufs[s][ci]
                                at, as0, _ = abufs[s][ai]
                                bj = int(pl.slot_base[s][j])
                                if "slotmm" in ab:
                                    continue
                                nc.tensor.matmul(
                                    out=pS[:, bj:bj + W],
                                    lhsT=et[:, j - es0, :], rhs=at[:, j - as0, :],
                                    start=False, stop=False,
                                    skip_group_check=True)
                        sS = sb.tile([P, RSLOT * P], bf, tag="sS")
                        nc.scalar.activation(out=sS[:], in_=pS[:],
                                             func=mybir.ActivationFunctionType.Copy)
                        if layer == 1:
                            pO = psl.tile([P, FEAT], f32, tag="pO", space="PSUM")
                            for r in range(RSLOT):
                                nc.tensor.matmul(out=pO[:],
                                                 lhsT=sS[:, r * P:(r + 1) * P],
                                                 rhs=ww[:, r * FEAT:(r + 1) * FEAT],
                                                 start=(r == 0), stop=False)
                            nc.tensor.matmul(out=pO[:], lhsT=ones1[:], rhs=bb[:],
                                             start=False, stop=True)
                            ho = sb.tile([P, FEAT], bf, tag="ho")
                            nc.scalar.activation(out=ho[:], in_=pO[:],
                                                 func=mybir.ActivationFunctionType.Copy)
                            nc.sync.dma_start(out_shard[t * P:(t + 1) * P, :], ho[:])
                            if t + 1 == pl.piece_tb[piece_of_tile[t] + 1]:
                                ag_piece(out_shard, out_halves,
                                         int(piece_of_tile[t]))
                        else:
                            pO = psl.tile([P, P], f32, tag="pO", space="PSUM")
                            for r in range(RSLOT):
                                nc.tensor.matmul(out=pO[:],
                                                 lhsT=ww[:, r * FEAT:(r + 1) * FEAT],
                                                 rhs=sS[:, r * P:(r + 1) * P],
                                                 start=(r == 0), stop=False)
                            nc.tensor.matmul(out=pO[:], lhsT=b2[:], rhs=ones1[:],
                                             start=False, stop=True)
                            h2T = sb.tile([P, P], bf, tag="h2T")
                            nc.vector.tensor_copy(out=h2T[:], in_=pO[:])
                            pL = psl.tile([CLASSES, P], f32, tag="pL", space="PSUM")
                            nc.tensor.matmul(out=pL[:], lhsT=fc3w[:], rhs=h2T[:],
                                             start=True, stop=False)
                            nc.tensor.matmul(out=pL[:], lhsT=fc3b[:], rhs=ones1[:],
                                             start=False, stop=True)
                            lg = sb.tile([CLASSES, P], f32, tag="lg")
                            nc.vector.tensor_copy(out=lg[:], in_=pL[:])
                            nc.sync.dma_start(p_logT[:, t * P:(t + 1) * P], lg[:])

            rgcn_layer(h_shard, h_half, ww1, b1, 1, h1_shard, h1_half)
            rgcn_layer(h1_shard, h1_half, ww2, b2, 2, None, None)

    nc.compile()
    return nc


# ============================ host packing =============================

def pack_inputs(pl, inputs):
    """Build per-core in_maps from the full problem inputs."""
    NS, NSP, NT = pl.NS, pl.NSP, pl.NT
    TC = TEXT // P

    vf = np.asarray(inputs["value_feature"], np.float32)
    tf = np.asarray(inputs["text_feature"], np.float32)

    def shard_textT(c):
        x = np.zeros((NSP, TEXT), np.float32)
        m = pl.node_of_row[c] >= 0
        x[m] = tf[pl.node_of_row[c][m]]
        # [NT, 128p(k within chunk), TC, 128n] -> flat [NT, 128, TC*128]
        y = x.reshape(NT, P, TC, P).transpose(0, 3, 2, 1)
        return np.ascontiguousarray(y.reshape(NT, P, TC * P).astype(BF16))

    def shard_valT(c):
        x = np.zeros((NSP, VAL), np.float32)
        m = pl.node_of_row[c] >= 0
        x[m] = vf[pl.node_of_row[c][m]]
        return np.ascontiguousarray(x.T.astype(BF16))

    f32 = np.float32
    fc1w = np.asarray(inputs["fc1_w"], f32)
    fc2w = np.asarray(inputs["fc2_w"], f32)
    relw = np.asarray(inputs["relu_w"], f32)
    beff = (np.concatenate([np.asarray(inputs["fc1_b"], f32),
                            np.asarray(inputs["fc2_b"], f32)]) @ relw
            + np.asarray(inputs["relu_b"], f32))
    # fold the relu layer into the fc projections (all linear):
    # h = lrelu(value @ (fc1 @ rw_v) + text @ (fc2 @ rw_t) + beff)
    fwv = fc1w @ relw[:FEAT]            # [16, 128]
    fwt = fc2w @ relw[FEAT:]            # [768, 128]
    # fwt host layout [128 k, TC*128 f]: [k, c*128+f] = fwt[c*128+k, f]
    fwt_t = np.ascontiguousarray(
        fwt.reshape(TC, P, FEAT).transpose(1, 0, 2).reshape(P, TC * FEAT).astype(BF16))

    def stack_w(wrel, wroot):
        w = np.concatenate([np.asarray(wrel, f32),
                            np.asarray(wroot, f32)[None]], 0)  # [4,128,128]
        return np.ascontiguousarray(w.transpose(1, 0, 2).reshape(P, RSLOT * FEAT).astype(BF16))

    ww1 = stack_w(inputs["rgcn1_wrel"], inputs["rgcn1_wroot"])
    ww2 = stack_w(inputs["rgcn2_wrel"], inputs["rgcn2_wroot"])

    layout, blob_n = blob_layout(pl)
    shared = dict(
        fwv=fwv.astype(BF16), fwt=fwt_t,
        beff=beff[None].astype(BF16),
        ww1=ww1, b1=np.asarray(inputs["rgcn1_b"], f32)[None].astype(BF16),
        ww2=ww2, b2=np.asarray(inputs["rgcn2_b"], f32)[None].astype(BF16),
        fc3w=np.asarray(inputs["fc3_w"], f32).astype(BF16),
        fc3b=np.asarray(inputs["fc3_b"], f32)[None].astype(BF16),
        iota16=np.tile(np.arange(W, dtype=f32), (P, ABATCH)).astype(BF16),
        ones1=np.ones((1, P), f32).astype(BF16),
        ident=np.eye(P, dtype=f32).astype(BF16),
    )

    in_maps = []
    for c in range(CORES):
        lo, hi = pl.idx_wrapped[c]
        vals = dict(shared)
        vals["textT"] = shard_textT(c)
        vals["valT"] = shard_valT(c)
        vals["idxlo"] = (wrap16(lo.reshape(-1)) if lo.size
                         else np.zeros((P, 8), np.int16)).view(BF16)
        vals["idxhi"] = (wrap16(hi.reshape(-1)) if hi.size
                         else np.zeros((P, 8), np.int16)).view(BF16)
        vals["keys"] = pl.keys[c] if pl.NSLOT else np.zeros((P, 1), BF16)
        vals["norms"] = pl.norms[c] if pl.NSLOT else np.zeros((P, 1), BF16)
        blob = np.zeros((1, blob_n), BF16)
        for name, (off, n, shape) in layout.items():
            a = vals[name]
            assert a.size == n, (name, a.shape, shape)
            blob[0, off:off + n] = a.reshape(-1)
        in_maps.append({"blob": blob})
    return in_maps


# ============================ entry point =============================

_cache = {}


def kernel(**inputs):
    ei = np.asarray(inputs["edge_index"], np.int64)
    et = np.asarray(inputs["edge_type"], np.int64)
    idx = np.asarray(inputs["idx"], np.int64)

    key = hash((ei.tobytes(), et.tobytes()))
    if key not in _cache:
        pl = make_plan(ei, et)
        nc = build_bass(pl)
        _cache[key] = (pl, nc)
    pl, nc = _cache[key]

    in_maps = pack_inputs(pl, inputs)
    res = run_bass_kernel_spmd(nc, in_maps, list(range(CORES)))

    logits = np.zeros((N_NODES, CLASSES), np.float32)
    for c in range(CORES):
        lt = res.results[c]["logitsT"]  # [2, NSP]
        m = pl.node_of_row[c] >= 0
        logits[pl.node_of_row[c][m]] = lt[:, m].T
    out = logits[idx]
    return out.astype(np.float32)
